# revision 12
# baseline (speedup 1.0000x reference)
"""DynamicConv (MoE-routed per-sample conv) Trainium2 kernel.

Problem (hardcoded — kernel.py must be self-contained):
  x      (64, 256, 1024, 1) f32
  cond   (64, 256)          f32
  w1     (64, 256)          f32   attention MLP layer 1  (HIDDEN=64, CS=256)
  w2     (4, 64)            f32   attention MLP layer 2  (K=4)
  weight (4, 256, 256, 3, 3) f32  K expert kernels (COUT, CIN, 3, 3)
  bias   (4, 256)           f32

  att    = softmax((relu(cond @ w1.T) @ w2.T) / 30)          (64, 4)
  agg_w  = einsum('bk,koihw->boihw', att, weight)
  agg_b  = att @ bias
  out[b] = conv2d(x[b], agg_w[b], stride 1, pad 1) + agg_b[b] (64, 256, 1024, 1)

Key algebraic facts used:
  * Input width is 1, so with padding (1,1) only the middle column
    (kw == 1) of each 3x3 kernel ever multiplies real data: the conv is a
    1-D conv over L with kernel 3 using weight[..., 1].
  * softmax weights sum to 1, so
      agg = sum_k a_k W_k = W_0 + sum_{k>=1} a_k (W_k - W_0),
    which needs only 3 fused (scale*D_k + acc) vector ops per sample.

Performance design (measured on the axon trn2 cores):
  * The whole conv datapath is fp16: the PE streams 16-bit at 2 rows/
    cycle (~157 TF/s, 2x the fp32r rate), so fp16 halves both the PE time
    and the HBM traffic vs the fp32r kernel. fp16's 10 mantissa bits keep
    the end-to-end rel err at ~8e-4 (gate is 2e-2); value ranges (|x|<6,
    |w|<0.3, |y|<4) are far from fp16 limits. PSUM accumulates fp32.
  * Engine budget per body: PE ~20.5us (roof), DVE mixing ~19.2us
    (3 STT passes x 8 samples, 2x mode), Act evictions ~13.7us. The DMA
    map keeps Act and DVE free of DMA issue work: SP carries x + routing
    smalls, gpsimd/SWDGE carries y + wpack + the att bounce.
  * wpool/rpool are double-buffered across repeat bodies (WBUFS=2) so
    body n+1's routing/weight loads overlap body n's conv tail.

Sharding: pure data-parallel over batch. 8 samples per NeuronCore, the
(small) expert kernels / MLP replicated; no cross-core communication.
"""

import os

import numpy as np

import concourse.bass as bass
import concourse.tile as tile
from concourse import bacc, mybir
from concourse.bass_utils import run_bass_kernel_spmd

B, CIN, COUT, CS, K, L = 64, 256, 256, 256, 4, 1024
HIDDEN = CS // 4
TEMPERATURE = 30.0
NCORES = 8
BLOC = B // NCORES  # samples per core

F32 = mybir.dt.float32
# fp16 datapath: the PE streams fp16 at the same 1 cycle/row as fp32r, so
# the conv matmuls cost the same — but x / weights / y move over HBM at
# half the bytes, which is what the fp32 kernel was bound on. fp16 (10
# mantissa bits) beats bf16 ~8x on quantization error and the value
# ranges here (|x|<6, |w|<0.3, |y|<4) are far from fp16 limits. PSUM
# still accumulates in fp32; the routing pipeline stays fp32 (tiny).
F16 = mybir.dt.float16


LAST_EXEC_TIME_NS = None
TRACE = os.environ.get("BASS_KERNEL_TRACE", "0") == "1"
# benchmark-only ablation knob (unused by the grading path): comma list of
# {"pe", "xdma", "ydma", "mix"} stages to skip when building the program.
_SKIP = frozenset(filter(None, os.environ.get("KERNEL_SKIP", "").split(",")))
if os.environ.get("KERNEL_BENCH") != "1":
    _SKIP = frozenset()  # ablations require explicit opt-in; grading path is full
# benchmark-only A/B knob: "1" = balanced DMA rings + gpsimd smalls (default)
_BAL = os.environ.get("KERNEL_DMA_BALANCE", "1") == "1"
# benchmark-only A/B knob: wpack bulk on the gpsimd SWDGE queue (3rd path)
_W2G = os.environ.get("KERNEL_WPACK_GPSIMD", "1") == "1"
_XBUFS = int(os.environ.get("KERNEL_XBUFS", "6"))
_OBUFS = int(os.environ.get("KERNEL_OBUFS", "6"))
_ABUFS = int(os.environ.get("KERNEL_ABUFS", "3"))
_Y2G = os.environ.get("KERNEL_Y_GPSIMD", "1") == "1"
_T5IN = os.environ.get("KERNEL_T5_INNER", "0") == "1"
_YGB = int(os.environ.get("KERNEL_YG_FROM", "6"))
# DMA map v2: the Activation and DVE engines issue ZERO DMAs (they are
# saturated by psum eviction / weight mixing); SP carries x + routing
# smalls (~565ns sequencer cost per DMA), gpsimd/SWDGE carries y + wpack
# + the att bounce (~25ns sequencer cost, generation on idle Q7 cores).
_V2 = os.environ.get("KERNEL_DMA_V2", "1") == "1"
# benchmark A/B: fuse x and y transfers to ~1MB per sample (one DMA each)
_BIG = os.environ.get("KERNEL_BIG_DMA", "1" if _V2 else "0") == "1"
# benchmark A/B: single-bank psum tiles with more bufs
_PS1 = os.environ.get("KERNEL_PS1", "1") == "1"
# benchmark A/B: issue y stores at elevated scheduler priority
_YPRIO = os.environ.get("KERNEL_YPRIO", "0") == "1"
# benchmark A/B: interleave the two t5 accumulation groups so consecutive
# matmuls share the same stationary lhsT (tests weight-load dedupe)
_T5X = os.environ.get("KERNEL_T5X", "0") == "1"
# benchmark A/B: shrink routing psum pool to 1 bank, deepen conv psum to 7
_PS7 = os.environ.get("KERNEL_PS7", "1") == "1"
# benchmark A/B: routing smalls ahead of wpack on the gpsimd queue; att
# bounce on a HW ring so it cannot stall the FIFO behind its data dep
_PRO = os.environ.get("KERNEL_PROLOGUE", "0") == "1"
# double-buffer resident weights / routing tiles across bodies: lets body
# n+1's routing/weight DMAs and matmuls overlap body n's conv tail
_WBUFS = int(os.environ.get("KERNEL_WBUFS", "2"))


def _build(nc: bass.Bass, repeat: int = 1):
    """Emit the single-core program (SPMD: every core runs this).

    repeat > 1 re-emits the whole body N times inside one NEFF — used only
    by the benchmark harness to measure steady-state body time without
    per-execution dispatch overhead."""
    if _BIG:
        x_d = nc.dram_tensor(
            "x", [BLOC, 128, 2 * (L + 2)], F16, kind="ExternalInput"
        ).ap()
    else:
        x_d = nc.dram_tensor(
            "x", [BLOC, 2, 128, L + 2], F16, kind="ExternalInput"
        ).ap()
    condt_d = nc.dram_tensor("condt", [2, 128, BLOC], F32, kind="ExternalInput").ap()
    w1t_d = nc.dram_tensor("w1t", [2, 128, HIDDEN], F32, kind="ExternalInput").ap()
    w2t_d = nc.dram_tensor("w2t", [HIDDEN, K], F32, kind="ExternalInput").ap()
    biask_d = nc.dram_tensor("biask", [K, COUT], F32, kind="ExternalInput").ap()
    # wpack[0] = W0, wpack[k] = W_k - W0 (k=1..3); layout [k][p][i2*768 + kh*256 + o]
    wpack_d = nc.dram_tensor("wpack", [K, 128, 1536], F16, kind="ExternalInput").ap()
    y_d = nc.dram_tensor("y", [BLOC, COUT, L], F16, kind="ExternalOutput").ap()

    from contextlib import ExitStack

    with tile.TileContext(nc) as tc, ExitStack() as ctx:
        pools = dict(
            wpool=ctx.enter_context(tc.tile_pool(name="wpool", bufs=_WBUFS)),
            rpool=ctx.enter_context(tc.tile_pool(name="rpool", bufs=_WBUFS)),
            rps=ctx.enter_context(tc.tile_pool(name="rps", bufs=(1 if _PS7 else 2), space="PSUM")),
            xpool=ctx.enter_context(tc.tile_pool(name="xpool", bufs=_XBUFS)),
            aggpool=ctx.enter_context(tc.tile_pool(name="aggpool", bufs=_ABUFS)),
            pspool=ctx.enter_context(tc.tile_pool(name="pspool", bufs=((7 if _PS7 else 6) if _PS1 else 3), space="PSUM")),
            opool=ctx.enter_context(tc.tile_pool(name="opool", bufs=_OBUFS)),
        )
        dram = dict(
            x_d=x_d, condt_d=condt_d, w1t_d=w1t_d, w2t_d=w2t_d,
            biask_d=biask_d, wpack_d=wpack_d, y_d=y_d,
        )
        for _rep in range(repeat):
            _emit_body(nc, tc, _rep, dram, pools)

    return nc


def _emit_body(nc, tc, _rep, dram, pools):
    x_d, condt_d, w1t_d = dram["x_d"], dram["condt_d"], dram["w1t_d"]
    w2t_d, biask_d, wpack_d, y_d = (
        dram["w2t_d"], dram["biask_d"], dram["wpack_d"], dram["y_d"],
    )
    wpool, rpool, rps, xpool = (
        pools["wpool"], pools["rpool"], pools["rps"], pools["xpool"],
    )
    aggpool, pspool, opool = pools["aggpool"], pools["pspool"], pools["opool"]

    # ---- routing inputs + resident weights --------------------------
    sm = nc.sync if _V2 else (nc.gpsimd if _BAL else nc.sync)

    def _load_smalls():
        condt = rpool.tile([128, 2 * BLOC], F32, tag="condt")
        w1t = rpool.tile([128, 2 * HIDDEN], F32, tag="w1t")
        w2t = rpool.tile([HIDDEN, K], F32, tag="w2t")
        biask = rpool.tile([K, COUT], F32, tag="biask")
        for i2 in range(2):
            sm.dma_start(condt[:, i2 * BLOC:(i2 + 1) * BLOC], condt_d[i2])
            sm.dma_start(w1t[:, i2 * HIDDEN:(i2 + 1) * HIDDEN], w1t_d[i2])
        sm.dma_start(w2t[:], w2t_d[:])
        sm.dma_start(biask[:], biask_d[:])
        return condt, w1t, w2t, biask

    if _PRO:  # smalls drain the FIFO before the 3.15MB weight pack
        condt, w1t, w2t, biask = _load_smalls()

    wt = []
    for k in range(K):
        t = wpool.tile([128, 1536], F16, tag=f"wt{k}")
        (nc.gpsimd if (_V2 or _W2G)
         else (nc.sync if (k % 2 == 0 or not _BAL) else nc.scalar)
         ).dma_start(t[:], wpack_d[k])
        wt.append(t)

    if not _PRO:
        condt, w1t, w2t, biask = _load_smalls()

    psh = rps.tile([HIDDEN, BLOC], F32, tag="rp")  # hT = w1 @ cond_loc.T
    for i2 in range(2):
        nc.tensor.matmul(
            psh[:],
            lhsT=w1t[:, i2 * HIDDEN:(i2 + 1) * HIDDEN],
            rhs=condt[:, i2 * BLOC:(i2 + 1) * BLOC],
            start=(i2 == 0),
            stop=(i2 == 1),
        )
    ht = rpool.tile([HIDDEN, BLOC], F32)
    nc.scalar.activation(ht[:], psh[:], mybir.ActivationFunctionType.Relu)

    psl = rps.tile([BLOC, K], F32, tag="rp")  # logits (b, k)
    nc.tensor.matmul(psl[:], lhsT=ht[:], rhs=w2t[:])
    # stable softmax: e = exp((l - max)/T); bias = -max/T per-partition
    lmax = rpool.tile([BLOC, 1], F32)
    nc.vector.tensor_reduce(lmax[:], psl[:], mybir.AxisListType.X, mybir.AluOpType.max)
    nmax = rpool.tile([BLOC, 1], F32)
    nc.scalar.mul(nmax[:], lmax[:], -1.0 / TEMPERATURE)
    e = rpool.tile([BLOC, K], F32)
    nc.scalar.activation(
        e[:], psl[:], mybir.ActivationFunctionType.Exp,
        bias=nmax[:], scale=1.0 / TEMPERATURE,
    )
    ssum = rpool.tile([BLOC, 1], F32)
    nc.vector.tensor_reduce(ssum[:], e[:], mybir.AxisListType.X, mybir.AluOpType.add)
    rcp = rpool.tile([BLOC, 1], F32)
    nc.vector.reciprocal(rcp[:], ssum[:])
    att = rpool.tile([BLOC, K], F32)
    nc.vector.tensor_scalar(att[:], e[:], rcp[:], None, mybir.AluOpType.mult)

    # att (8p, 4f) -> attrow (1, 32) and attT (4, 8) via a DRAM bounce
    # (partition-crossing SBUF->SBUF DMA trips the sim's conflict checker)
    att_scr = nc.dram_tensor(f"att_scr{_rep}", [BLOC, K], F32).ap()
    bounce = nc.gpsimd if _V2 else (nc.sync if _PRO else sm)
    bounce.dma_start(att_scr[:], att[:])
    attrow = rpool.tile([1, BLOC * K], F32)
    bounce.dma_start(attrow[:], att_scr.rearrange("b k -> (b k)"))
    attT = rpool.tile([K, BLOC], F32)
    (nc.gpsimd if _V2 else (nc.scalar if _PRO else sm)).dma_start(
        attT[:], att_scr.rearrange("b k -> k b"))

    # broadcast att across all 128 partitions: ones(1,128).T @ attrow(1,32)
    ones = rpool.tile([1, 128], F32)
    nc.vector.memset(ones[:], 1.0)
    psbc = rps.tile([128, BLOC * K], F32, tag="rp")
    nc.tensor.matmul(psbc[:], lhsT=ones[:], rhs=attrow[:])
    # fp16 so the mixing STT sees all-2-byte sources (keeps DVE 2x mode)
    attbc = rpool.tile([128, BLOC * K], F16)
    nc.scalar.copy(attbc[:], psbc[:])

    # aggregated bias, transposed: aggbT[o, (o2, b)] = sum_k bias[k, o] att[b, k]
    aggbT = rpool.tile([128, 2 * BLOC], F32)
    for o2 in range(2):
        psb = rps.tile([128, BLOC], F32, tag="rp")
        nc.tensor.matmul(psb[:], lhsT=biask[:, o2 * 128:(o2 + 1) * 128], rhs=attT[:])
        nc.scalar.copy(aggbT[:, o2 * BLOC:(o2 + 1) * BLOC], psb[:])

    # ---- per-sample: mix weights, conv, bias, store -----------------
    for b in range(BLOC):
        # padded input tiles, one per 128-channel chunk
        if _BIG:
            xt = xpool.tile([128, 2 * (L + 2)], F16, tag="xp0")
            if "xdma" not in _SKIP:
                (nc.sync if (_V2 or b % 2 == 0 or not _BAL)
                 else nc.scalar).dma_start(xt[:], x_d[b])
            else:
                nc.vector.memset(xt[:, 0:1].bitcast(mybir.dt.uint16), 0)
            xp = [xt[:, 0:L + 2], xt[:, L + 2:2 * (L + 2)]]
        else:
            xp = []
            for i2 in range(2):
                t = xpool.tile([128, L + 2], F16, tag=f"xp{i2}")
                if "xdma" not in _SKIP:
                    eng = (nc.sync if i2 == 0 else nc.gpsimd) if _V2 else (
                        nc.sync if (i2 == 0 or not _BAL) else nc.scalar)
                    eng.dma_start(t[:], x_d[b, i2])
                else:  # ablation: mark tile written so Tile allocates it
                    nc.vector.memset(t[:, 0:1].bitcast(mybir.dt.uint16), 0)
                xp.append(t)

        # agg = W0 + a1*D1 + a2*D2 + a3*D3   (3 fused DVE ops, in place)
        ag = aggpool.tile([128, 1536], F16)
        sc = lambda k: attbc[:, b * K + k:b * K + k + 1]
        nc.vector.scalar_tensor_tensor(
            ag[:], wt[1][:], sc(1), wt[0][:],
            mybir.AluOpType.mult, mybir.AluOpType.add,
        )
        if "mix" not in _SKIP:  # ablation: "mix" keeps only the first op
            nc.vector.scalar_tensor_tensor(
                ag[:], wt[2][:], sc(2), ag[:],
                mybir.AluOpType.mult, mybir.AluOpType.add,
            )
            nc.vector.scalar_tensor_tensor(
                ag[:], wt[3][:], sc(3), ag[:],
                mybir.AluOpType.mult, mybir.AluOpType.add,
            )

        if _BIG:
            osty = opool.tile([128, 2 * L], F16, tag="osty")
        else:
            osty = None
        for o2 in range(2):
            ost = osty[:, o2 * L:(o2 + 1) * L] if _BIG else opool.tile(
                [128, L], F16, tag="ost"
            )
            if "pe" in _SKIP:
                if "ydma" not in _SKIP:
                    (nc.scalar if o2 == 0 else nc.sync).dma_start(
                        y_d[b, o2 * 128:(o2 + 1) * 128, :], ost[:]
                    )
                continue
            if not _PS1:
                ps = pspool.tile([128, L], F32, tag="ps")  # spans 2 PSUM banks
            if _PS1:
                for t5 in range(2):
                    ps1t = pspool.tile([128, 512], F32, tag="ps1t")
                    n_mm = 0
                    n_tot = 1 if "pelite" in _SKIP else 6
                    for i2 in range(2):
                        for kh in range(3):
                            if n_mm >= n_tot:
                                continue
                            nc.tensor.matmul(
                                ps1t[:],
                                lhsT=ag[
                                    :,
                                    i2 * 768 + kh * 256 + o2 * 128:
                                    i2 * 768 + kh * 256 + o2 * 128 + 128,
                                ],
                                rhs=xp[i2][:, kh + t5 * 512:kh + t5 * 512 + 512],
                                start=(n_mm == 0),
                                stop=(n_mm == n_tot - 1),
                            )
                            n_mm += 1
                    nc.scalar.activation(
                        ost[:, t5 * 512:(t5 + 1) * 512],
                        ps1t[:],
                        mybir.ActivationFunctionType.Identity,
                        bias=aggbT[:, o2 * BLOC + b:o2 * BLOC + b + 1],
                        scale=1.0,
                    )
            elif _T5IN:
                # both L-halves of each stationary lhsT back-to-back
                for i2 in range(2):
                    for kh in range(3):
                        for t5 in range(2):
                            nc.tensor.matmul(
                                ps[:, t5 * 512:(t5 + 1) * 512],
                                lhsT=ag[
                                    :,
                                    i2 * 768 + kh * 256 + o2 * 128:
                                    i2 * 768 + kh * 256 + o2 * 128 + 128,
                                ],
                                rhs=xp[i2][:, kh + t5 * 512:kh + t5 * 512 + 512],
                                start=(i2 == 0 and kh == 0),
                                stop=(i2 == 1 and kh == 2),
                                skip_group_check=True,
                            )
            else:
                for t5 in range(2):  # accumulation group per 512-wide bank
                    n_mm = 0
                    for i2 in range(2):
                        for kh in range(3):
                            nc.tensor.matmul(
                                ps[:, t5 * 512:(t5 + 1) * 512],
                                lhsT=ag[
                                    :,
                                    i2 * 768 + kh * 256 + o2 * 128:
                                    i2 * 768 + kh * 256 + o2 * 128 + 128,
                                ],
                                rhs=xp[i2][:, kh + t5 * 512:kh + t5 * 512 + 512],
                                start=(n_mm == 0),
                                stop=(n_mm == 5),
                            )
                            n_mm += 1
            if not _PS1:
                # evict both banks + fused per-(b,o) bias add
                nc.scalar.activation(
                    ost[:],
                    ps[:],
                    mybir.ActivationFunctionType.Identity,
                    bias=aggbT[:, o2 * BLOC + b:o2 * BLOC + b + 1],
                    scale=1.0,
                )
            if _BIG:
                continue
            if "ydma" not in _SKIP:
                if _V2:
                    yeng = nc.sync if o2 == 0 else nc.gpsimd
                else:
                    yeng = nc.scalar if (o2 == 0 or not _BAL) else nc.sync
                    if _Y2G and b >= _YGB:
                        yeng = nc.gpsimd
                if _YPRIO:
                    with tc.high_priority():
                        yeng.dma_start(y_d[b, o2 * 128:(o2 + 1) * 128, :], ost[:])
                else:
                    yeng.dma_start(y_d[b, o2 * 128:(o2 + 1) * 128, :], ost[:])
        if _BIG and "ydma" not in _SKIP and "pe" not in _SKIP:
            if _V2:
                yeng = nc.gpsimd
            else:
                yeng = nc.scalar if b % 2 == 0 else nc.sync
                if _Y2G and b >= _YGB:
                    yeng = nc.gpsimd
            yeng.dma_start(
                y_d[b].rearrange("(o2 p) h -> p o2 h", o2=2), osty[:]
            )


def _prep_shared(cond, w1, w2, weight, bias):
    """Host-side layout prep for the replicated tensors."""
    wm = weight[:, :, :, :, 1]  # (K, COUT, CIN, 3) — only kw==1 touches data
    # device layout: [k][p][i2*768 + kh*256 + o], i = i2*128 + p
    wdev = (
        wm.transpose(2, 3, 1, 0)  # (CIN, 3, COUT, K)
        .reshape(2, 128, 3, COUT, K)
        .transpose(4, 1, 0, 2, 3)  # (K, 128, 2, 3, COUT)
        .reshape(K, 128, 1536)
    )
    wpack = wdev.copy()
    wpack[1:] -= wpack[0:1]  # difference trick
    condt = np.ascontiguousarray(cond.T).reshape(2, 128, B)
    w1t = np.ascontiguousarray(w1.T).reshape(2, 128, HIDDEN)
    w2t = np.ascontiguousarray(w2.T)
    return (
        np.ascontiguousarray(wpack).astype(np.float16),
        condt,
        w1t,
        w2t,
        np.ascontiguousarray(bias),
    )


_CACHED_NC = None


def _get_nc():
    global _CACHED_NC
    if _CACHED_NC is None:
        nc = bacc.Bacc(
            "TRN2",
            target_bir_lowering=False,
            debug=False,
            enable_asserts=True,
            num_devices=NCORES,
        )
        _build(nc)
        nc.compile()
        _CACHED_NC = nc
    return _CACHED_NC


def _make_in_maps(inputs):
    x = np.asarray(inputs["x"], dtype=np.float32)
    cond = np.asarray(inputs["cond"], dtype=np.float32)
    w1 = np.asarray(inputs["w1"], dtype=np.float32)
    w2 = np.asarray(inputs["w2"], dtype=np.float32)
    weight = np.asarray(inputs["weight"], dtype=np.float32)
    bias = np.asarray(inputs["bias"], dtype=np.float32)

    wpack, condt, w1t, w2t, biask = _prep_shared(cond, w1, w2, weight, bias)
    xr = x.reshape(B, CIN, L).astype(np.float16)
    if _BIG:
        xpad = np.zeros((B, 128, 2, L + 2), np.float16)
        xpad[:, :, :, 1:L + 1] = xr.reshape(B, 2, 128, L).transpose(0, 2, 1, 3)
        xpad = xpad.reshape(B, 128, 2 * (L + 2))
    else:
        xpad = np.zeros((B, 2, 128, L + 2), np.float16)
        xpad[:, :, :, 1:L + 1] = xr.reshape(B, 2, 128, L)

    in_maps = []
    for c in range(NCORES):
        sl = slice(c * BLOC, (c + 1) * BLOC)
        in_maps.append(
            {
                "x": np.ascontiguousarray(xpad[sl]),
                "condt": np.ascontiguousarray(condt[:, :, sl]),
                "w1t": w1t,
                "w2t": w2t,
                "biask": biask,
                "wpack": wpack,
            }
        )
    return in_maps


def kernel(x, cond, w1, w2, weight, bias):
    global LAST_EXEC_TIME_NS
    in_maps = _make_in_maps(
        {"x": x, "cond": cond, "w1": w1, "w2": w2, "weight": weight, "bias": bias}
    )
    nc = _get_nc()
    res = run_bass_kernel_spmd(
        nc, in_maps, core_ids=list(range(NCORES)), trace=TRACE
    )
    LAST_EXEC_TIME_NS = res.exec_time_ns

    y = np.concatenate([res.results[c]["y"] for c in range(NCORES)], axis=0)
    return y.reshape(B, COUT, L, 1).astype(np.float32)



# revision 15
# speedup vs baseline: 1.2451x; 1.2451x over previous
"""DynamicConv (MoE-routed per-sample conv) Trainium2 kernel.

Problem (hardcoded — kernel.py must be self-contained):
  x      (64, 256, 1024, 1) f32
  cond   (64, 256)          f32
  w1     (64, 256)          f32   attention MLP layer 1  (HIDDEN=64, CS=256)
  w2     (4, 64)            f32   attention MLP layer 2  (K=4)
  weight (4, 256, 256, 3, 3) f32  K expert kernels (COUT, CIN, 3, 3)
  bias   (4, 256)           f32

  att    = softmax((relu(cond @ w1.T) @ w2.T) / 30)          (64, 4)
  agg_w  = einsum('bk,koihw->boihw', att, weight)
  agg_b  = att @ bias
  out[b] = conv2d(x[b], agg_w[b], stride 1, pad 1) + agg_b[b] (64, 256, 1024, 1)

Key algebraic facts used:
  * Input width is 1, so with padding (1,1) only the middle column
    (kw == 1) of each 3x3 kernel ever multiplies real data: the conv is a
    1-D conv over L with kernel 3 using weight[..., 1].
  * softmax weights sum to 1, so
      agg = sum_k a_k W_k = W_0 + sum_{k>=1} a_k (W_k - W_0),
    which needs only 3 fused (scale*D_k + acc) vector ops per sample.

Performance design (measured on the axon trn2 cores):
  * The whole conv datapath is fp16: the PE streams 16-bit at 2 rows/
    cycle (~157 TF/s, 2x the fp32r rate), so fp16 halves both the PE time
    and the HBM traffic vs the fp32r kernel. fp16's 10 mantissa bits keep
    the end-to-end rel err at ~8e-4 (gate is 2e-2); value ranges (|x|<6,
    |w|<0.3, |y|<4) are far from fp16 limits. PSUM accumulates fp32.
  * Engine budget per body: PE ~20.5us (roof), DVE mixing ~19.2us
    (3 STT passes x 8 samples, 2x mode), Act evictions ~13.7us. The DMA
    map keeps Act and DVE free of DMA issue work: SP carries x + routing
    smalls, gpsimd/SWDGE carries y + wpack + the att bounce.
  * wpool/rpool are double-buffered across repeat bodies (WBUFS=2) so
    body n+1's routing/weight loads overlap body n's conv tail.

Sharding: pure data-parallel over batch. 8 samples per NeuronCore, the
(small) expert kernels / MLP replicated; no cross-core communication.
"""

import os

import numpy as np

import concourse.bass as bass
import concourse.tile as tile
from concourse import bacc, mybir
from concourse.bass_utils import run_bass_kernel_spmd

B, CIN, COUT, CS, K, L = 64, 256, 256, 256, 4, 1024
HIDDEN = CS // 4
TEMPERATURE = 30.0
NCORES = 8
BLOC = B // NCORES  # samples per core

F32 = mybir.dt.float32
# fp16 datapath: the PE streams fp16 at the same 1 cycle/row as fp32r, so
# the conv matmuls cost the same — but x / weights / y move over HBM at
# half the bytes, which is what the fp32 kernel was bound on. fp16 (10
# mantissa bits) beats bf16 ~8x on quantization error and the value
# ranges here (|x|<6, |w|<0.3, |y|<4) are far from fp16 limits. PSUM
# still accumulates in fp32; the routing pipeline stays fp32 (tiny).
F16 = mybir.dt.float16


LAST_EXEC_TIME_NS = None
TRACE = os.environ.get("BASS_KERNEL_TRACE", "0") == "1"
# benchmark-only ablation knob (unused by the grading path): comma list of
# {"pe", "xdma", "ydma", "mix"} stages to skip when building the program.
_SKIP = frozenset(filter(None, os.environ.get("KERNEL_SKIP", "").split(",")))
if os.environ.get("KERNEL_BENCH") != "1":
    _SKIP = frozenset()  # ablations require explicit opt-in; grading path is full
# benchmark-only A/B knob: "1" = balanced DMA rings + gpsimd smalls (default)
_BAL = os.environ.get("KERNEL_DMA_BALANCE", "1") == "1"
# benchmark-only A/B knob: wpack bulk on the gpsimd SWDGE queue (3rd path)
_W2G = os.environ.get("KERNEL_WPACK_GPSIMD", "1") == "1"
_XBUFS = int(os.environ.get("KERNEL_XBUFS", "6"))
_OBUFS = int(os.environ.get("KERNEL_OBUFS", "6"))
_ABUFS = int(os.environ.get("KERNEL_ABUFS", "3"))
_Y2G = os.environ.get("KERNEL_Y_GPSIMD", "1") == "1"
_T5IN = os.environ.get("KERNEL_T5_INNER", "0") == "1"
_YGB = int(os.environ.get("KERNEL_YG_FROM", "6"))
# DMA map v2: the Activation and DVE engines issue ZERO DMAs (they are
# saturated by psum eviction / weight mixing); SP carries x + routing
# smalls (~565ns sequencer cost per DMA), gpsimd/SWDGE carries y + wpack
# + the att bounce (~25ns sequencer cost, generation on idle Q7 cores).
_V2 = os.environ.get("KERNEL_DMA_V2", "1") == "1"
# benchmark A/B: fuse x and y transfers to ~1MB per sample (one DMA each)
_BIG = os.environ.get("KERNEL_BIG_DMA", "1" if _V2 else "0") == "1"
# benchmark A/B: single-bank psum tiles with more bufs
_PS1 = os.environ.get("KERNEL_PS1", "1") == "1"
# benchmark A/B: issue y stores at elevated scheduler priority
_YPRIO = os.environ.get("KERNEL_YPRIO", "0") == "1"
# benchmark A/B: interleave the two t5 accumulation groups so consecutive
# matmuls share the same stationary lhsT (tests weight-load dedupe)
_T5X = os.environ.get("KERNEL_T5X", "0") == "1"
# benchmark A/B: att transpose/broadcast on-chip (PE transpose + per-k
# broadcast matmuls) instead of the DRAM bounce, removing a DRAM
# roundtrip from the PE's in-order critical path at each body boundary
_NOB = os.environ.get("KERNEL_NOBOUNCE", "0") == "1"
# benchmark A/B: shrink routing psum pool to 1 bank, deepen conv psum to 7
_PS7 = os.environ.get("KERNEL_PS7", "1") == "1"
# benchmark A/B: routing smalls ahead of wpack on the gpsimd queue; att
# bounce on a HW ring so it cannot stall the FIFO behind its data dep
_PRO = os.environ.get("KERNEL_PROLOGUE", "0") == "1"
# double-buffer resident weights / routing tiles across bodies: lets body
# n+1's routing/weight DMAs and matmuls overlap body n's conv tail
_WBUFS = int(os.environ.get("KERNEL_WBUFS", "2"))


def _build(nc: bass.Bass, repeat: int = 1):
    """Emit the single-core program (SPMD: every core runs this).

    repeat > 1 re-emits the whole body N times inside one NEFF — used only
    by the benchmark harness to measure steady-state body time without
    per-execution dispatch overhead."""
    if _BIG:
        x_d = nc.dram_tensor(
            "x", [BLOC, 128, 2 * (L + 2)], F16, kind="ExternalInput"
        ).ap()
    else:
        x_d = nc.dram_tensor(
            "x", [BLOC, 2, 128, L + 2], F16, kind="ExternalInput"
        ).ap()
    condt_d = nc.dram_tensor("condt", [2, 128, BLOC], F32, kind="ExternalInput").ap()
    w1t_d = nc.dram_tensor("w1t", [2, 128, HIDDEN], F32, kind="ExternalInput").ap()
    w2t_d = nc.dram_tensor("w2t", [HIDDEN, K], F32, kind="ExternalInput").ap()
    biask_d = nc.dram_tensor("biask", [K, COUT], F32, kind="ExternalInput").ap()
    eye_d = nc.dram_tensor("eye", [BLOC, BLOC], F32, kind="ExternalInput").ap()
    biaskp_d = nc.dram_tensor("biaskp", [128, COUT], F32, kind="ExternalInput").ap()
    # wpack[0] = W0, wpack[k] = W_k - W0 (k=1..3); layout [k][p][i2*768 + kh*256 + o]
    wpack_d = nc.dram_tensor("wpack", [K, 128, 1536], F16, kind="ExternalInput").ap()
    y_d = nc.dram_tensor("y", [BLOC, COUT, L], F16, kind="ExternalOutput").ap()

    from contextlib import ExitStack

    with tile.TileContext(nc) as tc, ExitStack() as ctx:
        pools = dict(
            wpool=ctx.enter_context(tc.tile_pool(name="wpool", bufs=_WBUFS)),
            rpool=ctx.enter_context(tc.tile_pool(name="rpool", bufs=_WBUFS)),
            rps=ctx.enter_context(tc.tile_pool(name="rps", bufs=(1 if _PS7 else 2), space="PSUM")),
            xpool=ctx.enter_context(tc.tile_pool(name="xpool", bufs=_XBUFS)),
            aggpool=ctx.enter_context(tc.tile_pool(name="aggpool", bufs=_ABUFS)),
            pspool=ctx.enter_context(tc.tile_pool(name="pspool", bufs=((7 if _PS7 else 6) if _PS1 else 3), space="PSUM")),
            opool=ctx.enter_context(tc.tile_pool(name="opool", bufs=_OBUFS)),
        )
        dram = dict(
            x_d=x_d, condt_d=condt_d, w1t_d=w1t_d, w2t_d=w2t_d,
            biask_d=biask_d, wpack_d=wpack_d, y_d=y_d, eye_d=eye_d,
            biaskp_d=biaskp_d,
        )
        for _rep in range(repeat):
            _emit_body(nc, tc, _rep, dram, pools)

    return nc


def _emit_body(nc, tc, _rep, dram, pools):
    x_d, condt_d, w1t_d = dram["x_d"], dram["condt_d"], dram["w1t_d"]
    w2t_d, biask_d, wpack_d, y_d = (
        dram["w2t_d"], dram["biask_d"], dram["wpack_d"], dram["y_d"],
    )
    eye_d = dram["eye_d"]
    biaskp_d = dram["biaskp_d"]
    wpool, rpool, rps, xpool = (
        pools["wpool"], pools["rpool"], pools["rps"], pools["xpool"],
    )
    aggpool, pspool, opool = pools["aggpool"], pools["pspool"], pools["opool"]

    # ---- routing inputs + resident weights --------------------------
    sm = nc.sync if _V2 else (nc.gpsimd if _BAL else nc.sync)

    def _load_smalls():
        condt = rpool.tile([128, 2 * BLOC], F32, tag="condt")
        w1t = rpool.tile([128, 2 * HIDDEN], F32, tag="w1t")
        w2t = rpool.tile([HIDDEN, K], F32, tag="w2t")
        biask = rpool.tile([K, COUT], F32, tag="biask")
        for i2 in range(2):
            sm.dma_start(condt[:, i2 * BLOC:(i2 + 1) * BLOC], condt_d[i2])
            sm.dma_start(w1t[:, i2 * HIDDEN:(i2 + 1) * HIDDEN], w1t_d[i2])
        sm.dma_start(w2t[:], w2t_d[:])
        sm.dma_start(biask[:], biask_d[:])
        return condt, w1t, w2t, biask

    if _PRO:  # smalls drain the FIFO before the 3.15MB weight pack
        condt, w1t, w2t, biask = _load_smalls()

    wt = []
    for k in range(K):
        t = wpool.tile([128, 1536], F16, tag=f"wt{k}")
        (nc.gpsimd if (_V2 or _W2G)
         else (nc.sync if (k % 2 == 0 or not _BAL) else nc.scalar)
         ).dma_start(t[:], wpack_d[k])
        wt.append(t)

    if not _PRO:
        condt, w1t, w2t, biask = _load_smalls()

    psh = rps.tile([HIDDEN, BLOC], F32, tag="rp")  # hT = w1 @ cond_loc.T
    for i2 in range(2):
        nc.tensor.matmul(
            psh[:],
            lhsT=w1t[:, i2 * HIDDEN:(i2 + 1) * HIDDEN],
            rhs=condt[:, i2 * BLOC:(i2 + 1) * BLOC],
            start=(i2 == 0),
            stop=(i2 == 1),
        )
    ht = rpool.tile([HIDDEN, BLOC], F32)
    nc.scalar.activation(ht[:], psh[:], mybir.ActivationFunctionType.Relu)

    psl = rps.tile([BLOC, K], F32, tag="rp")  # logits (b, k)
    nc.tensor.matmul(psl[:], lhsT=ht[:], rhs=w2t[:])
    # stable softmax: e = exp((l - max)/T); bias = -max/T per-partition
    lmax = rpool.tile([BLOC, 1], F32)
    nc.vector.tensor_reduce(lmax[:], psl[:], mybir.AxisListType.X, mybir.AluOpType.max)
    nmax = rpool.tile([BLOC, 1], F32)
    nc.scalar.mul(nmax[:], lmax[:], -1.0 / TEMPERATURE)
    e = rpool.tile([BLOC, K], F32)
    nc.scalar.activation(
        e[:], psl[:], mybir.ActivationFunctionType.Exp,
        bias=nmax[:], scale=1.0 / TEMPERATURE,
    )
    ssum = rpool.tile([BLOC, 1], F32)
    nc.vector.tensor_reduce(ssum[:], e[:], mybir.AxisListType.X, mybir.AluOpType.add)
    rcp = rpool.tile([BLOC, 1], F32)
    nc.vector.reciprocal(rcp[:], ssum[:])
    if _NOB:
        # att columns for k=0..2 at free offsets {0,32,64} so the transpose
        # lands each k-row on a legal matmul base partition; k=3 transposes
        # separately onto partition 0 of its own tile.
        att_pad = rpool.tile([BLOC, 65], F32)
        nc.vector.memset(att_pad[:], 0.0)
        nc.vector.tensor_scalar(
            att_pad[:, 0:65:32], e[:, 0:3], rcp[:], None, mybir.AluOpType.mult)
        att_c3 = rpool.tile([BLOC, 1], F32)
        nc.vector.tensor_scalar(
            att_c3[:], e[:, 3:4], rcp[:], None, mybir.AluOpType.mult)
    else:
        att = rpool.tile([BLOC, K], F32)
        nc.vector.tensor_scalar(att[:], e[:], rcp[:], None, mybir.AluOpType.mult)

    # fp16 so the mixing STT sees all-2-byte sources (keeps DVE 2x mode)
    attbc = rpool.tile([128, BLOC * K], F16)
    if _NOB:
        # On-chip att transpose: att_pad (8p, 97f with data at f=32k) ->
        # psT (97p, 8f) puts row k at partition 32k, a legal matmul base.
        # Broadcast + bias-mix then use [1,...] slices at those bases; no
        # DRAM roundtrip on the PE's in-order critical path.
        ones = rpool.tile([65, 128], F32)
        nc.vector.memset(ones[:], 1.0)
        eye = rpool.tile([BLOC, BLOC], F32, tag="eye")
        sm.dma_start(eye[:], eye_d[:])
        # biaskp rows: 0 <- bias[3], 32 <- bias[1], 64 <- bias[2]
        biaskp = rpool.tile([128, COUT], F32, tag="biaskp")
        sm.dma_start(biaskp[:], biaskp_d[:])
        psT = rps.tile([65, BLOC], F32, tag="rp")
        nc.tensor.matmul(psT[:], lhsT=att_pad[:], rhs=eye[:], is_transpose=True)
        attP = rpool.tile([65, BLOC], F32)
        nc.scalar.copy(attP[:], psT[:])
        psT3 = rps.tile([1, BLOC], F32, tag="rp")
        nc.tensor.matmul(psT3[:], lhsT=att_c3[:], rhs=eye[:], is_transpose=True)
        attP3 = rpool.tile([1, BLOC], F32)
        nc.scalar.copy(attP3[:], psT3[:])
        krhs = [attP[0:1, :], attP[32:33, :], attP[64:65, :], attP3[0:1, :]]
        kbase = [0, 32, 64, 0]
        psbc = rps.tile([128, BLOC * K], F32, tag="rp")
        for k in range(K):
            nc.tensor.matmul(
                psbc[:, k * BLOC:(k + 1) * BLOC],
                lhsT=ones[kbase[k]:kbase[k] + 1, :],
                rhs=krhs[k],
            )
        # mixing scalars live k-major here: attbc[:, k*BLOC + b]
        nc.scalar.copy(attbc[:], psbc[:])
        aggbT = rpool.tile([128, 2 * BLOC], F32)
        klhs_row = [None, 32, 64, 0]  # None -> biask row 0 (bias[0])
        for o2 in range(2):
            psb = rps.tile([128, BLOC], F32, tag="rp")
            for k in range(K):
                lt = (biask[0:1, o2 * 128:(o2 + 1) * 128] if klhs_row[k] is None
                      else biaskp[klhs_row[k]:klhs_row[k] + 1,
                                  o2 * 128:(o2 + 1) * 128])
                nc.tensor.matmul(
                    psb[:], lhsT=lt, rhs=krhs[k],
                    start=(k == 0), stop=(k == K - 1),
                )
            nc.scalar.copy(aggbT[:, o2 * BLOC:(o2 + 1) * BLOC], psb[:])
    else:
        ones = rpool.tile([1, 128], F32)
        nc.vector.memset(ones[:], 1.0)
        attT = rpool.tile([K, BLOC], F32)
        # att (8p, 4f) -> attrow (1, 32) and attT (4, 8) via a DRAM bounce
        # (partition-crossing SBUF->SBUF DMA trips the sim's conflict checker)
        att_scr = nc.dram_tensor(f"att_scr{_rep}", [BLOC, K], F32).ap()
        bounce = nc.gpsimd if _V2 else (nc.sync if _PRO else sm)
        bounce.dma_start(att_scr[:], att[:])
        attrow = rpool.tile([1, BLOC * K], F32)
        bounce.dma_start(attrow[:], att_scr.rearrange("b k -> (b k)"))
        (nc.gpsimd if _V2 else (nc.scalar if _PRO else sm)).dma_start(
            attT[:], att_scr.rearrange("b k -> k b"))
        psbc = rps.tile([128, BLOC * K], F32, tag="rp")
        nc.tensor.matmul(psbc[:], lhsT=ones[:], rhs=attrow[:])
        nc.scalar.copy(attbc[:], psbc[:])

    if not _NOB:
        # aggregated bias, transposed: aggbT[o, (o2,b)] = sum_k bias[k,o] att[b,k]
        aggbT = rpool.tile([128, 2 * BLOC], F32)
        for o2 in range(2):
            psb = rps.tile([128, BLOC], F32, tag="rp")
            nc.tensor.matmul(
                psb[:], lhsT=biask[:, o2 * 128:(o2 + 1) * 128], rhs=attT[:])
            nc.scalar.copy(aggbT[:, o2 * BLOC:(o2 + 1) * BLOC], psb[:])

    # ---- per-sample: mix weights, conv, bias, store -----------------
    for b in range(BLOC):
        # padded input tiles, one per 128-channel chunk
        if _BIG:
            xt = xpool.tile([128, 2 * (L + 2)], F16, tag="xp0")
            if "xdma" not in _SKIP:
                (nc.sync if (_V2 or b % 2 == 0 or not _BAL)
                 else nc.scalar).dma_start(xt[:], x_d[b])
            else:
                nc.vector.memset(xt[:, 0:1].bitcast(mybir.dt.uint16), 0)
            xp = [xt[:, 0:L + 2], xt[:, L + 2:2 * (L + 2)]]
        else:
            xp = []
            for i2 in range(2):
                t = xpool.tile([128, L + 2], F16, tag=f"xp{i2}")
                if "xdma" not in _SKIP:
                    eng = (nc.sync if i2 == 0 else nc.gpsimd) if _V2 else (
                        nc.sync if (i2 == 0 or not _BAL) else nc.scalar)
                    eng.dma_start(t[:], x_d[b, i2])
                else:  # ablation: mark tile written so Tile allocates it
                    nc.vector.memset(t[:, 0:1].bitcast(mybir.dt.uint16), 0)
                xp.append(t)

        # agg = W0 + a1*D1 + a2*D2 + a3*D3   (3 fused DVE ops, in place)
        ag = aggpool.tile([128, 1536], F16)
        sc = (lambda k: attbc[:, k * BLOC + b:k * BLOC + b + 1]) if _NOB else (
            lambda k: attbc[:, b * K + k:b * K + k + 1])
        nc.vector.scalar_tensor_tensor(
            ag[:], wt[1][:], sc(1), wt[0][:],
            mybir.AluOpType.mult, mybir.AluOpType.add,
        )
        if "mix" not in _SKIP:  # ablation: "mix" keeps only the first op
            nc.vector.scalar_tensor_tensor(
                ag[:], wt[2][:], sc(2), ag[:],
                mybir.AluOpType.mult, mybir.AluOpType.add,
            )
            nc.vector.scalar_tensor_tensor(
                ag[:], wt[3][:], sc(3), ag[:],
                mybir.AluOpType.mult, mybir.AluOpType.add,
            )

        if _BIG:
            osty = opool.tile([128, 2 * L], F16, tag="osty")
        else:
            osty = None
        for o2 in range(2):
            ost = osty[:, o2 * L:(o2 + 1) * L] if _BIG else opool.tile(
                [128, L], F16, tag="ost"
            )
            if "pe" in _SKIP:
                if "ydma" not in _SKIP:
                    (nc.scalar if o2 == 0 else nc.sync).dma_start(
                        y_d[b, o2 * 128:(o2 + 1) * 128, :], ost[:]
                    )
                continue
            if not _PS1:
                ps = pspool.tile([128, L], F32, tag="ps")  # spans 2 PSUM banks
            if _PS1:
                for t5 in range(2):
                    ps1t = pspool.tile([128, 512], F32, tag="ps1t")
                    n_mm = 0
                    n_tot = 1 if "pelite" in _SKIP else 6
                    for i2 in range(2):
                        for kh in range(3):
                            if n_mm >= n_tot:
                                continue
                            nc.tensor.matmul(
                                ps1t[:],
                                lhsT=ag[
                                    :,
                                    i2 * 768 + kh * 256 + o2 * 128:
                                    i2 * 768 + kh * 256 + o2 * 128 + 128,
                                ],
                                rhs=xp[i2][:, kh + t5 * 512:kh + t5 * 512 + 512],
                                start=(n_mm == 0),
                                stop=(n_mm == n_tot - 1),
                            )
                            n_mm += 1
                    nc.scalar.activation(
                        ost[:, t5 * 512:(t5 + 1) * 512],
                        ps1t[:],
                        mybir.ActivationFunctionType.Identity,
                        bias=aggbT[:, o2 * BLOC + b:o2 * BLOC + b + 1],
                        scale=1.0,
                    )
            elif _T5IN:
                # both L-halves of each stationary lhsT back-to-back
                for i2 in range(2):
                    for kh in range(3):
                        for t5 in range(2):
                            nc.tensor.matmul(
                                ps[:, t5 * 512:(t5 + 1) * 512],
                                lhsT=ag[
                                    :,
                                    i2 * 768 + kh * 256 + o2 * 128:
                                    i2 * 768 + kh * 256 + o2 * 128 + 128,
                                ],
                                rhs=xp[i2][:, kh + t5 * 512:kh + t5 * 512 + 512],
                                start=(i2 == 0 and kh == 0),
                                stop=(i2 == 1 and kh == 2),
                                skip_group_check=True,
                            )
            else:
                for t5 in range(2):  # accumulation group per 512-wide bank
                    n_mm = 0
                    for i2 in range(2):
                        for kh in range(3):
                            nc.tensor.matmul(
                                ps[:, t5 * 512:(t5 + 1) * 512],
                                lhsT=ag[
                                    :,
                                    i2 * 768 + kh * 256 + o2 * 128:
                                    i2 * 768 + kh * 256 + o2 * 128 + 128,
                                ],
                                rhs=xp[i2][:, kh + t5 * 512:kh + t5 * 512 + 512],
                                start=(n_mm == 0),
                                stop=(n_mm == 5),
                            )
                            n_mm += 1
            if not _PS1:
                # evict both banks + fused per-(b,o) bias add
                nc.scalar.activation(
                    ost[:],
                    ps[:],
                    mybir.ActivationFunctionType.Identity,
                    bias=aggbT[:, o2 * BLOC + b:o2 * BLOC + b + 1],
                    scale=1.0,
                )
            if _BIG:
                continue
            if "ydma" not in _SKIP:
                if _V2:
                    yeng = nc.sync if o2 == 0 else nc.gpsimd
                else:
                    yeng = nc.scalar if (o2 == 0 or not _BAL) else nc.sync
                    if _Y2G and b >= _YGB:
                        yeng = nc.gpsimd
                if _YPRIO:
                    with tc.high_priority():
                        yeng.dma_start(y_d[b, o2 * 128:(o2 + 1) * 128, :], ost[:])
                else:
                    yeng.dma_start(y_d[b, o2 * 128:(o2 + 1) * 128, :], ost[:])
        if _BIG and "ydma" not in _SKIP and "pe" not in _SKIP:
            if _V2:
                yeng = nc.gpsimd
            else:
                yeng = nc.scalar if b % 2 == 0 else nc.sync
                if _Y2G and b >= _YGB:
                    yeng = nc.gpsimd
            yeng.dma_start(
                y_d[b].rearrange("(o2 p) h -> p o2 h", o2=2), osty[:]
            )


def _prep_shared(cond, w1, w2, weight, bias):
    """Host-side layout prep for the replicated tensors."""
    wm = weight[:, :, :, :, 1]  # (K, COUT, CIN, 3) — only kw==1 touches data
    # device layout: [k][p][i2*768 + kh*256 + o], i = i2*128 + p
    wdev = (
        wm.transpose(2, 3, 1, 0)  # (CIN, 3, COUT, K)
        .reshape(2, 128, 3, COUT, K)
        .transpose(4, 1, 0, 2, 3)  # (K, 128, 2, 3, COUT)
        .reshape(K, 128, 1536)
    )
    wpack = wdev.copy()
    wpack[1:] -= wpack[0:1]  # difference trick
    condt = np.ascontiguousarray(cond.T).reshape(2, 128, B)
    w1t = np.ascontiguousarray(w1.T).reshape(2, 128, HIDDEN)
    w2t = np.ascontiguousarray(w2.T)
    return (
        np.ascontiguousarray(wpack).astype(np.float16),
        condt,
        w1t,
        w2t,
        np.ascontiguousarray(bias),
    )


_CACHED_NC = None


def _get_nc():
    global _CACHED_NC
    if _CACHED_NC is None:
        nc = bacc.Bacc(
            "TRN2",
            target_bir_lowering=False,
            debug=False,
            enable_asserts=True,
            num_devices=NCORES,
        )
        _build(nc)
        nc.compile()
        _CACHED_NC = nc
    return _CACHED_NC


def _make_in_maps(inputs):
    x = np.asarray(inputs["x"], dtype=np.float32)
    cond = np.asarray(inputs["cond"], dtype=np.float32)
    w1 = np.asarray(inputs["w1"], dtype=np.float32)
    w2 = np.asarray(inputs["w2"], dtype=np.float32)
    weight = np.asarray(inputs["weight"], dtype=np.float32)
    bias = np.asarray(inputs["bias"], dtype=np.float32)

    wpack, condt, w1t, w2t, biask = _prep_shared(cond, w1, w2, weight, bias)
    biaskp = np.zeros((128, COUT), np.float32)
    biaskp[0] = biask[3]
    biaskp[32] = biask[1]
    biaskp[64] = biask[2]
    xr = x.reshape(B, CIN, L).astype(np.float16)
    if _BIG:
        xpad = np.zeros((B, 128, 2, L + 2), np.float16)
        xpad[:, :, :, 1:L + 1] = xr.reshape(B, 2, 128, L).transpose(0, 2, 1, 3)
        xpad = xpad.reshape(B, 128, 2 * (L + 2))
    else:
        xpad = np.zeros((B, 2, 128, L + 2), np.float16)
        xpad[:, :, :, 1:L + 1] = xr.reshape(B, 2, 128, L)

    in_maps = []
    for c in range(NCORES):
        sl = slice(c * BLOC, (c + 1) * BLOC)
        in_maps.append(
            {
                "x": np.ascontiguousarray(xpad[sl]),
                "condt": np.ascontiguousarray(condt[:, :, sl]),
                "w1t": w1t,
                "w2t": w2t,
                "biask": biask,
                "wpack": wpack,
                "eye": np.eye(BLOC, dtype=np.float32),
                "biaskp": biaskp,
            }
        )
    return in_maps


def kernel(x, cond, w1, w2, weight, bias):
    global LAST_EXEC_TIME_NS
    in_maps = _make_in_maps(
        {"x": x, "cond": cond, "w1": w1, "w2": w2, "weight": weight, "bias": bias}
    )
    nc = _get_nc()
    res = run_bass_kernel_spmd(
        nc, in_maps, core_ids=list(range(NCORES)), trace=TRACE
    )
    LAST_EXEC_TIME_NS = res.exec_time_ns

    y = np.concatenate([res.results[c]["y"] for c in range(NCORES)], axis=0)
    return y.reshape(B, COUT, L, 1).astype(np.float32)



# revision 19
# speedup vs baseline: 2.4427x; 1.9620x over previous
"""DynamicConv (MoE-routed per-sample conv) Trainium2 kernel.

Problem (hardcoded — kernel.py must be self-contained):
  x      (64, 256, 1024, 1) f32
  cond   (64, 256)          f32
  w1     (64, 256)          f32   attention MLP layer 1  (HIDDEN=64, CS=256)
  w2     (4, 64)            f32   attention MLP layer 2  (K=4)
  weight (4, 256, 256, 3, 3) f32  K expert kernels (COUT, CIN, 3, 3)
  bias   (4, 256)           f32

  att    = softmax((relu(cond @ w1.T) @ w2.T) / 30)          (64, 4)
  agg_w  = einsum('bk,koihw->boihw', att, weight)
  agg_b  = att @ bias
  out[b] = conv2d(x[b], agg_w[b], stride 1, pad 1) + agg_b[b] (64, 256, 1024, 1)

Key algebraic facts used:
  * Input width is 1, so with padding (1,1) only the middle column
    (kw == 1) of each 3x3 kernel ever multiplies real data: the conv is a
    1-D conv over L with kernel 3 using weight[..., 1].
  * softmax weights sum to 1, so
      agg = sum_k a_k W_k = W_0 + sum_{k>=1} a_k (W_k - W_0),
    which needs only 3 fused (scale*D_k + acc) vector ops per sample.

Performance design (measured on the axon trn2 cores):
  * The whole conv datapath is fp16: the PE streams 16-bit at 2 rows/
    cycle (~157 TF/s, 2x the fp32r rate), so fp16 halves both the PE time
    and the HBM traffic vs the fp32r kernel. fp16's 10 mantissa bits keep
    the end-to-end rel err at ~8e-4 (gate is 2e-2); value ranges (|x|<6,
    |w|<0.3, |y|<4) are far from fp16 limits. PSUM accumulates fp32.
  * Engine budget per body: PE ~20.5us (roof), DVE mixing ~19.2us
    (3 STT passes x 8 samples, 2x mode), Act evictions ~13.7us. The DMA
    map keeps Act and DVE free of DMA issue work: SP carries x + routing
    smalls, gpsimd/SWDGE carries y + wpack + the att bounce.
  * wpool/rpool are double-buffered across repeat bodies (WBUFS=2) so
    body n+1's routing/weight loads overlap body n's conv tail.

Sharding: pure data-parallel over batch. 8 samples per NeuronCore, the
(small) expert kernels / MLP replicated; no cross-core communication.
"""

import os

import numpy as np

import concourse.bass as bass
import concourse.tile as tile
from concourse import bacc, mybir
from concourse.bass_utils import run_bass_kernel_spmd

B, CIN, COUT, CS, K, L = 64, 256, 256, 256, 4, 1024
HIDDEN = CS // 4
TEMPERATURE = 30.0
NCORES = 8
BLOC = B // NCORES  # samples per core

F32 = mybir.dt.float32
# fp16 datapath: the PE streams fp16 at the same 1 cycle/row as fp32r, so
# the conv matmuls cost the same — but x / weights / y move over HBM at
# half the bytes, which is what the fp32 kernel was bound on. fp16 (10
# mantissa bits) beats bf16 ~8x on quantization error and the value
# ranges here (|x|<6, |w|<0.3, |y|<4) are far from fp16 limits. PSUM
# still accumulates in fp32; the routing pipeline stays fp32 (tiny).
F16 = mybir.dt.float16


LAST_EXEC_TIME_NS = None
TRACE = os.environ.get("BASS_KERNEL_TRACE", "0") == "1"
# benchmark-only ablation knob (unused by the grading path): comma list of
# {"pe", "xdma", "ydma", "mix"} stages to skip when building the program.
_SKIP = frozenset(filter(None, os.environ.get("KERNEL_SKIP", "").split(",")))
if os.environ.get("KERNEL_BENCH") != "1":
    _SKIP = frozenset()  # ablations require explicit opt-in; grading path is full
# benchmark-only A/B knob: "1" = balanced DMA rings + gpsimd smalls (default)
_BAL = os.environ.get("KERNEL_DMA_BALANCE", "1") == "1"
# benchmark-only A/B knob: wpack bulk on the gpsimd SWDGE queue (3rd path)
_W2G = os.environ.get("KERNEL_WPACK_GPSIMD", "1") == "1"
_XBUFS = int(os.environ.get("KERNEL_XBUFS", "6"))
_OBUFS = int(os.environ.get("KERNEL_OBUFS", "6"))
_ABUFS = int(os.environ.get("KERNEL_ABUFS", "3"))
_Y2G = os.environ.get("KERNEL_Y_GPSIMD", "1") == "1"
_T5IN = os.environ.get("KERNEL_T5_INNER", "0") == "1"
_YGB = int(os.environ.get("KERNEL_YG_FROM", "6"))
# DMA map v2: the Activation and DVE engines issue ZERO DMAs (they are
# saturated by psum eviction / weight mixing); SP carries x + routing
# smalls (~565ns sequencer cost per DMA), gpsimd/SWDGE carries y + wpack
# + the att bounce (~25ns sequencer cost, generation on idle Q7 cores).
_V2 = os.environ.get("KERNEL_DMA_V2", "1") == "1"
# benchmark A/B: fuse x and y transfers to ~1MB per sample (one DMA each)
_BIG = os.environ.get("KERNEL_BIG_DMA", "1" if _V2 else "0") == "1"
# benchmark A/B: single-bank psum tiles with more bufs
_PS1 = os.environ.get("KERNEL_PS1", "1") == "1"
# benchmark A/B: issue y stores at elevated scheduler priority
_YPRIO = os.environ.get("KERNEL_YPRIO", "0") == "1"
# benchmark A/B: interleave the two t5 accumulation groups so consecutive
# matmuls share the same stationary lhsT (tests weight-load dedupe)
_T5X = os.environ.get("KERNEL_T5X", "0") == "1"
# benchmark A/B: emit the routing chain (psh..aggbT) at high scheduler
# priority so body n+1's softmax/bounce ops hoist early into body n's
# in-order engine queues, killing the PE stall on psbc at each boundary
_RPRIO = os.environ.get("KERNEL_RPRIO", "1") == "1"
# benchmark A/B: att transpose/broadcast on-chip (PE transpose + per-k
# broadcast matmuls) instead of the DRAM bounce, removing a DRAM
# roundtrip from the PE's in-order critical path at each body boundary
_NOB = os.environ.get("KERNEL_NOBOUNCE", "0") == "1"
# benchmark A/B: shrink routing psum pool to 1 bank, deepen conv psum to 7
_PS7 = os.environ.get("KERNEL_PS7", "1") == "1"
# benchmark A/B: routing smalls ahead of wpack on the gpsimd queue; att
# bounce on a HW ring so it cannot stall the FIFO behind its data dep
_PRO = os.environ.get("KERNEL_PROLOGUE", "0") == "1"
# double-buffer resident weights / routing tiles across bodies: lets body
# n+1's routing/weight DMAs and matmuls overlap body n's conv tail
_WBUFS = int(os.environ.get("KERNEL_WBUFS", "2"))


def _build(nc: bass.Bass, repeat: int = 1):
    """Emit the single-core program (SPMD: every core runs this).

    repeat > 1 re-emits the whole body N times inside one NEFF — used only
    by the benchmark harness to measure steady-state body time without
    per-execution dispatch overhead."""
    if _BIG:
        x_d = nc.dram_tensor(
            "x", [BLOC, 128, 2 * (L + 2)], F16, kind="ExternalInput"
        ).ap()
    else:
        x_d = nc.dram_tensor(
            "x", [BLOC, 2, 128, L + 2], F16, kind="ExternalInput"
        ).ap()
    condt_d = nc.dram_tensor("condt", [2, 128, BLOC], F32, kind="ExternalInput").ap()
    w1t_d = nc.dram_tensor("w1t", [2, 128, HIDDEN], F32, kind="ExternalInput").ap()
    w2t_d = nc.dram_tensor("w2t", [HIDDEN, K], F32, kind="ExternalInput").ap()
    biask_d = nc.dram_tensor("biask", [K, COUT], F32, kind="ExternalInput").ap()
    eye_d = nc.dram_tensor("eye", [BLOC, BLOC], F16, kind="ExternalInput").ap()
    biaskp_d = nc.dram_tensor("biaskp", [128, 2 * COUT], F16, kind="ExternalInput").ap()
    # wpack[0] = W0, wpack[k] = W_k - W0 (k=1..3); layout [k][p][i2*768 + kh*256 + o]
    wpack_d = nc.dram_tensor("wpack", [K, 128, 1536], F16, kind="ExternalInput").ap()
    y_d = nc.dram_tensor("y", [BLOC, COUT, L], F16, kind="ExternalOutput").ap()

    from contextlib import ExitStack

    with tile.TileContext(nc) as tc, ExitStack() as ctx:
        pools = dict(
            wpool=ctx.enter_context(tc.tile_pool(name="wpool", bufs=_WBUFS)),
            rpool=ctx.enter_context(tc.tile_pool(name="rpool", bufs=_WBUFS)),
            rps=ctx.enter_context(tc.tile_pool(name="rps", bufs=(1 if _PS7 else 2), space="PSUM")),
            xpool=ctx.enter_context(tc.tile_pool(name="xpool", bufs=_XBUFS)),
            aggpool=ctx.enter_context(tc.tile_pool(name="aggpool", bufs=_ABUFS)),
            pspool=ctx.enter_context(tc.tile_pool(name="pspool", bufs=((7 if _PS7 else 6) if _PS1 else 3), space="PSUM")),
            opool=ctx.enter_context(tc.tile_pool(name="opool", bufs=_OBUFS)),
        )
        dram = dict(
            x_d=x_d, condt_d=condt_d, w1t_d=w1t_d, w2t_d=w2t_d,
            biask_d=biask_d, wpack_d=wpack_d, y_d=y_d, eye_d=eye_d,
            biaskp_d=biaskp_d,
        )
        for _rep in range(repeat):
            _emit_body(nc, tc, _rep, dram, pools)

    return nc


def _emit_body(nc, tc, _rep, dram, pools):
    x_d, condt_d, w1t_d = dram["x_d"], dram["condt_d"], dram["w1t_d"]
    w2t_d, biask_d, wpack_d, y_d = (
        dram["w2t_d"], dram["biask_d"], dram["wpack_d"], dram["y_d"],
    )
    eye_d = dram["eye_d"]
    biaskp_d = dram["biaskp_d"]
    wpool, rpool, rps, xpool = (
        pools["wpool"], pools["rpool"], pools["rps"], pools["xpool"],
    )
    aggpool, pspool, opool = pools["aggpool"], pools["pspool"], pools["opool"]

    # ---- routing inputs + resident weights --------------------------
    sm = nc.sync if _V2 else (nc.gpsimd if _BAL else nc.sync)

    def _load_smalls():
        condt = rpool.tile([128, 2 * BLOC], F32, tag="condt")
        w1t = rpool.tile([128, 2 * HIDDEN], F32, tag="w1t")
        w2t = rpool.tile([HIDDEN, K], F32, tag="w2t")
        biask = rpool.tile([K, COUT], F32, tag="biask")
        for i2 in range(2):
            sm.dma_start(condt[:, i2 * BLOC:(i2 + 1) * BLOC], condt_d[i2])
            sm.dma_start(w1t[:, i2 * HIDDEN:(i2 + 1) * HIDDEN], w1t_d[i2])
        sm.dma_start(w2t[:], w2t_d[:])
        sm.dma_start(biask[:], biask_d[:])
        return condt, w1t, w2t, biask

    if _PRO:  # smalls drain the FIFO before the 3.15MB weight pack
        condt, w1t, w2t, biask = _load_smalls()

    wt = []
    for k in range(K):
        t = wpool.tile([128, 1536], F16, tag=f"wt{k}")
        (nc.gpsimd if (_V2 or _W2G)
         else (nc.sync if (k % 2 == 0 or not _BAL) else nc.scalar)
         ).dma_start(t[:], wpack_d[k])
        wt.append(t)

    if not _PRO:
        condt, w1t, w2t, biask = _load_smalls()

    _rp_ctx = tc.high_priority() if _RPRIO else None
    if _rp_ctx is not None:
        _rp_ctx.__enter__()
    psh = rps.tile([HIDDEN, BLOC], F32, tag="rp")  # hT = w1 @ cond_loc.T
    for i2 in range(2):
        nc.tensor.matmul(
            psh[:],
            lhsT=w1t[:, i2 * HIDDEN:(i2 + 1) * HIDDEN],
            rhs=condt[:, i2 * BLOC:(i2 + 1) * BLOC],
            start=(i2 == 0),
            stop=(i2 == 1),
        )
    ht = rpool.tile([HIDDEN, BLOC], F32)
    nc.scalar.activation(ht[:], psh[:], mybir.ActivationFunctionType.Relu)

    psl = rps.tile([BLOC, K], F32, tag="rp")  # logits (b, k)
    nc.tensor.matmul(psl[:], lhsT=ht[:], rhs=w2t[:])
    # stable softmax: e = exp((l - max)/T); bias = -max/T per-partition
    lmax = rpool.tile([BLOC, 1], F32)
    nc.vector.tensor_reduce(lmax[:], psl[:], mybir.AxisListType.X, mybir.AluOpType.max)
    nmax = rpool.tile([BLOC, 1], F32)
    nc.scalar.mul(nmax[:], lmax[:], -1.0 / TEMPERATURE)
    e = rpool.tile([BLOC, K], F32)
    nc.scalar.activation(
        e[:], psl[:], mybir.ActivationFunctionType.Exp,
        bias=nmax[:], scale=1.0 / TEMPERATURE,
    )
    ssum = rpool.tile([BLOC, 1], F32)
    nc.vector.tensor_reduce(ssum[:], e[:], mybir.AxisListType.X, mybir.AluOpType.add)
    rcp = rpool.tile([BLOC, 1], F32)
    nc.vector.reciprocal(rcp[:], ssum[:])
    if _NOB:
        # att columns for k=0..2 at free offsets {0,32,64} so the transpose
        # lands each k-row on a legal matmul base partition; k=3 transposes
        # separately onto partition 0 of its own tile.
        att_pad = rpool.tile([BLOC, 65], F16)
        nc.vector.memset(att_pad[:], 0.0)
        nc.vector.tensor_scalar(
            att_pad[:, 0:65:32], e[:, 0:3], rcp[:], None, mybir.AluOpType.mult)
        att_c3 = rpool.tile([BLOC, 1], F16)
        nc.vector.tensor_scalar(
            att_c3[:], e[:, 3:4], rcp[:], None, mybir.AluOpType.mult)
    else:
        att = rpool.tile([BLOC, K], F32)
        nc.vector.tensor_scalar(att[:], e[:], rcp[:], None, mybir.AluOpType.mult)

    # fp16 so the mixing STT sees all-2-byte sources (keeps DVE 2x mode)
    attbc = rpool.tile([128, BLOC * K], F16)
    if _NOB:
        # On-chip att transpose: att_pad (8p, 97f with data at f=32k) ->
        # psT (97p, 8f) puts row k at partition 32k, a legal matmul base.
        # Broadcast + bias-mix then use [1,...] slices at those bases; no
        # DRAM roundtrip on the PE's in-order critical path.
        ones = rpool.tile([65, 128], F16)
        nc.vector.memset(ones[:], 1.0)
        eye = rpool.tile([BLOC, BLOC], F16, tag="eye")
        sm.dma_start(eye[:], eye_d[:])
        # biaskp rows: [r, 0:COUT] 0<-bias[3], 32<-bias[1], 64<-bias[2];
        # [0, COUT:2*COUT] <- bias[0] (k=0 needs a second base-0 lhsT row)
        biaskp = rpool.tile([128, 2 * COUT], F16, tag="biaskp")
        sm.dma_start(biaskp[:], biaskp_d[:])
        psT = rps.tile([65, BLOC], F16, tag="rp")
        nc.tensor.matmul(psT[:], lhsT=att_pad[:], rhs=eye[:], is_transpose=True)
        attP = rpool.tile([65, BLOC], F16)
        nc.scalar.copy(attP[:], psT[:])
        psT3 = rps.tile([1, BLOC], F16, tag="rp")
        nc.tensor.matmul(psT3[:], lhsT=att_c3[:], rhs=eye[:], is_transpose=True)
        attP3 = rpool.tile([1, BLOC], F16)
        nc.scalar.copy(attP3[:], psT3[:])
        krhs = [attP[0:1, :], attP[32:33, :], attP[64:65, :], attP3[0:1, :]]
        kbase = [0, 32, 64, 0]
        psbc = rps.tile([128, BLOC * K], F32, tag="rp")
        for k in range(K):
            nc.tensor.matmul(
                psbc[:, k * BLOC:(k + 1) * BLOC],
                lhsT=ones[kbase[k]:kbase[k] + 1, :],
                rhs=krhs[k],
            )
        # mixing scalars live k-major here: attbc[:, k*BLOC + b]
        nc.scalar.copy(attbc[:], psbc[:])
        aggbT = rpool.tile([128, 2 * BLOC], F32)
        klhs = [(0, COUT), (32, 0), (64, 0), (0, 0)]  # (row, col offset)
        for o2 in range(2):
            psb = rps.tile([128, BLOC], F32, tag="rp")
            for k in range(K):
                r, co = klhs[k]
                lt = biaskp[r:r + 1, co + o2 * 128:co + (o2 + 1) * 128]
                nc.tensor.matmul(
                    psb[:], lhsT=lt, rhs=krhs[k],
                    start=(k == 0), stop=(k == K - 1),
                )
            nc.scalar.copy(aggbT[:, o2 * BLOC:(o2 + 1) * BLOC], psb[:])
    else:
        ones = rpool.tile([1, 128], F32)
        nc.vector.memset(ones[:], 1.0)
        attT = rpool.tile([K, BLOC], F32)
        # att (8p, 4f) -> attrow (1, 32) and attT (4, 8) via a DRAM bounce
        # (partition-crossing SBUF->SBUF DMA trips the sim's conflict checker)
        att_scr = nc.dram_tensor(f"att_scr{_rep}", [BLOC, K], F32).ap()
        bounce = nc.gpsimd if _V2 else (nc.sync if _PRO else sm)
        bounce.dma_start(att_scr[:], att[:])
        attrow = rpool.tile([1, BLOC * K], F32)
        bounce.dma_start(attrow[:], att_scr.rearrange("b k -> (b k)"))
        (nc.gpsimd if _V2 else (nc.scalar if _PRO else sm)).dma_start(
            attT[:], att_scr.rearrange("b k -> k b"))
        psbc = rps.tile([128, BLOC * K], F32, tag="rp")
        nc.tensor.matmul(psbc[:], lhsT=ones[:], rhs=attrow[:])
        nc.scalar.copy(attbc[:], psbc[:])

    if not _NOB:
        # aggregated bias, transposed: aggbT[o, (o2,b)] = sum_k bias[k,o] att[b,k]
        aggbT = rpool.tile([128, 2 * BLOC], F32)
        for o2 in range(2):
            psb = rps.tile([128, BLOC], F32, tag="rp")
            nc.tensor.matmul(
                psb[:], lhsT=biask[:, o2 * 128:(o2 + 1) * 128], rhs=attT[:])
            nc.scalar.copy(aggbT[:, o2 * BLOC:(o2 + 1) * BLOC], psb[:])

    if _rp_ctx is not None:
        _rp_ctx.__exit__(None, None, None)

    # ---- per-sample: mix weights, conv, bias, store -----------------
    for b in range(BLOC):
        # padded input tiles, one per 128-channel chunk
        if _BIG:
            xt = xpool.tile([128, 2 * (L + 2)], F16, tag="xp0")
            if "xdma" not in _SKIP:
                (nc.sync if (_V2 or b % 2 == 0 or not _BAL)
                 else nc.scalar).dma_start(xt[:], x_d[b])
            else:
                nc.vector.memset(xt[:, 0:1].bitcast(mybir.dt.uint16), 0)
            xp = [xt[:, 0:L + 2], xt[:, L + 2:2 * (L + 2)]]
        else:
            xp = []
            for i2 in range(2):
                t = xpool.tile([128, L + 2], F16, tag=f"xp{i2}")
                if "xdma" not in _SKIP:
                    eng = (nc.sync if i2 == 0 else nc.gpsimd) if _V2 else (
                        nc.sync if (i2 == 0 or not _BAL) else nc.scalar)
                    eng.dma_start(t[:], x_d[b, i2])
                else:  # ablation: mark tile written so Tile allocates it
                    nc.vector.memset(t[:, 0:1].bitcast(mybir.dt.uint16), 0)
                xp.append(t)

        # agg = W0 + a1*D1 + a2*D2 + a3*D3   (3 fused DVE ops, in place)
        ag = aggpool.tile([128, 1536], F16)
        sc = (lambda k: attbc[:, k * BLOC + b:k * BLOC + b + 1]) if _NOB else (
            lambda k: attbc[:, b * K + k:b * K + k + 1])
        nc.vector.scalar_tensor_tensor(
            ag[:], wt[1][:], sc(1), wt[0][:],
            mybir.AluOpType.mult, mybir.AluOpType.add,
        )
        if "mix" not in _SKIP:  # ablation: "mix" keeps only the first op
            nc.vector.scalar_tensor_tensor(
                ag[:], wt[2][:], sc(2), ag[:],
                mybir.AluOpType.mult, mybir.AluOpType.add,
            )
            nc.vector.scalar_tensor_tensor(
                ag[:], wt[3][:], sc(3), ag[:],
                mybir.AluOpType.mult, mybir.AluOpType.add,
            )

        if _BIG:
            osty = opool.tile([128, 2 * L], F16, tag="osty")
        else:
            osty = None
        for o2 in range(2):
            ost = osty[:, o2 * L:(o2 + 1) * L] if _BIG else opool.tile(
                [128, L], F16, tag="ost"
            )
            if "pe" in _SKIP:
                if "ydma" not in _SKIP:
                    (nc.scalar if o2 == 0 else nc.sync).dma_start(
                        y_d[b, o2 * 128:(o2 + 1) * 128, :], ost[:]
                    )
                continue
            if not _PS1:
                ps = pspool.tile([128, L], F32, tag="ps")  # spans 2 PSUM banks
            if _PS1:
                for t5 in range(2):
                    ps1t = pspool.tile([128, 512], F32, tag="ps1t")
                    n_mm = 0
                    n_tot = 1 if "pelite" in _SKIP else 6
                    for i2 in range(2):
                        for kh in range(3):
                            if n_mm >= n_tot:
                                continue
                            nc.tensor.matmul(
                                ps1t[:],
                                lhsT=ag[
                                    :,
                                    i2 * 768 + kh * 256 + o2 * 128:
                                    i2 * 768 + kh * 256 + o2 * 128 + 128,
                                ],
                                rhs=xp[i2][:, kh + t5 * 512:kh + t5 * 512 + 512],
                                start=(n_mm == 0),
                                stop=(n_mm == n_tot - 1),
                            )
                            n_mm += 1
                    nc.scalar.activation(
                        ost[:, t5 * 512:(t5 + 1) * 512],
                        ps1t[:],
                        mybir.ActivationFunctionType.Identity,
                        bias=aggbT[:, o2 * BLOC + b:o2 * BLOC + b + 1],
                        scale=1.0,
                    )
            elif _T5IN:
                # both L-halves of each stationary lhsT back-to-back
                for i2 in range(2):
                    for kh in range(3):
                        for t5 in range(2):
                            nc.tensor.matmul(
                                ps[:, t5 * 512:(t5 + 1) * 512],
                                lhsT=ag[
                                    :,
                                    i2 * 768 + kh * 256 + o2 * 128:
                                    i2 * 768 + kh * 256 + o2 * 128 + 128,
                                ],
                                rhs=xp[i2][:, kh + t5 * 512:kh + t5 * 512 + 512],
                                start=(i2 == 0 and kh == 0),
                                stop=(i2 == 1 and kh == 2),
                                skip_group_check=True,
                            )
            else:
                for t5 in range(2):  # accumulation group per 512-wide bank
                    n_mm = 0
                    for i2 in range(2):
                        for kh in range(3):
                            nc.tensor.matmul(
                                ps[:, t5 * 512:(t5 + 1) * 512],
                                lhsT=ag[
                                    :,
                                    i2 * 768 + kh * 256 + o2 * 128:
                                    i2 * 768 + kh * 256 + o2 * 128 + 128,
                                ],
                                rhs=xp[i2][:, kh + t5 * 512:kh + t5 * 512 + 512],
                                start=(n_mm == 0),
                                stop=(n_mm == 5),
                            )
                            n_mm += 1
            if not _PS1:
                # evict both banks + fused per-(b,o) bias add
                nc.scalar.activation(
                    ost[:],
                    ps[:],
                    mybir.ActivationFunctionType.Identity,
                    bias=aggbT[:, o2 * BLOC + b:o2 * BLOC + b + 1],
                    scale=1.0,
                )
            if _BIG:
                continue
            if "ydma" not in _SKIP:
                if _V2:
                    yeng = nc.sync if o2 == 0 else nc.gpsimd
                else:
                    yeng = nc.scalar if (o2 == 0 or not _BAL) else nc.sync
                    if _Y2G and b >= _YGB:
                        yeng = nc.gpsimd
                if _YPRIO:
                    with tc.high_priority():
                        yeng.dma_start(y_d[b, o2 * 128:(o2 + 1) * 128, :], ost[:])
                else:
                    yeng.dma_start(y_d[b, o2 * 128:(o2 + 1) * 128, :], ost[:])
        if _BIG and "ydma" not in _SKIP and "pe" not in _SKIP:
            if _V2:
                yeng = nc.gpsimd
            else:
                yeng = nc.scalar if b % 2 == 0 else nc.sync
                if _Y2G and b >= _YGB:
                    yeng = nc.gpsimd
            yeng.dma_start(
                y_d[b].rearrange("(o2 p) h -> p o2 h", o2=2), osty[:]
            )


def _prep_shared(cond, w1, w2, weight, bias):
    """Host-side layout prep for the replicated tensors."""
    wm = weight[:, :, :, :, 1]  # (K, COUT, CIN, 3) — only kw==1 touches data
    # device layout: [k][p][i2*768 + kh*256 + o], i = i2*128 + p
    wdev = (
        wm.transpose(2, 3, 1, 0)  # (CIN, 3, COUT, K)
        .reshape(2, 128, 3, COUT, K)
        .transpose(4, 1, 0, 2, 3)  # (K, 128, 2, 3, COUT)
        .reshape(K, 128, 1536)
    )
    wpack = wdev.copy()
    wpack[1:] -= wpack[0:1]  # difference trick
    condt = np.ascontiguousarray(cond.T).reshape(2, 128, B)
    w1t = np.ascontiguousarray(w1.T).reshape(2, 128, HIDDEN)
    w2t = np.ascontiguousarray(w2.T)
    return (
        np.ascontiguousarray(wpack).astype(np.float16),
        condt,
        w1t,
        w2t,
        np.ascontiguousarray(bias),
    )


_CACHED_NC = None


def _get_nc():
    global _CACHED_NC
    if _CACHED_NC is None:
        nc = bacc.Bacc(
            "TRN2",
            target_bir_lowering=False,
            debug=False,
            enable_asserts=True,
            num_devices=NCORES,
        )
        _build(nc)
        nc.compile()
        _CACHED_NC = nc
    return _CACHED_NC


def _make_in_maps(inputs):
    x = np.asarray(inputs["x"], dtype=np.float32)
    cond = np.asarray(inputs["cond"], dtype=np.float32)
    w1 = np.asarray(inputs["w1"], dtype=np.float32)
    w2 = np.asarray(inputs["w2"], dtype=np.float32)
    weight = np.asarray(inputs["weight"], dtype=np.float32)
    bias = np.asarray(inputs["bias"], dtype=np.float32)

    wpack, condt, w1t, w2t, biask = _prep_shared(cond, w1, w2, weight, bias)
    biaskp = np.zeros((128, 2 * COUT), np.float16)
    biaskp[0, 0:COUT] = biask[3]
    biaskp[32, 0:COUT] = biask[1]
    biaskp[64, 0:COUT] = biask[2]
    biaskp[0, COUT:2 * COUT] = biask[0]
    xr = x.reshape(B, CIN, L).astype(np.float16)
    if _BIG:
        xpad = np.zeros((B, 128, 2, L + 2), np.float16)
        xpad[:, :, :, 1:L + 1] = xr.reshape(B, 2, 128, L).transpose(0, 2, 1, 3)
        xpad = xpad.reshape(B, 128, 2 * (L + 2))
    else:
        xpad = np.zeros((B, 2, 128, L + 2), np.float16)
        xpad[:, :, :, 1:L + 1] = xr.reshape(B, 2, 128, L)

    in_maps = []
    for c in range(NCORES):
        sl = slice(c * BLOC, (c + 1) * BLOC)
        in_maps.append(
            {
                "x": np.ascontiguousarray(xpad[sl]),
                "condt": np.ascontiguousarray(condt[:, :, sl]),
                "w1t": w1t,
                "w2t": w2t,
                "biask": biask,
                "wpack": wpack,
                "eye": np.eye(BLOC, dtype=np.float16),
                "biaskp": biaskp,
            }
        )
    return in_maps


def kernel(x, cond, w1, w2, weight, bias):
    global LAST_EXEC_TIME_NS
    in_maps = _make_in_maps(
        {"x": x, "cond": cond, "w1": w1, "w2": w2, "weight": weight, "bias": bias}
    )
    nc = _get_nc()
    res = run_bass_kernel_spmd(
        nc, in_maps, core_ids=list(range(NCORES)), trace=TRACE
    )
    LAST_EXEC_TIME_NS = res.exec_time_ns

    y = np.concatenate([res.results[c]["y"] for c in range(NCORES)], axis=0)
    return y.reshape(B, COUT, L, 1).astype(np.float32)

